# revision 2
# baseline (speedup 1.0000x reference)
"""Multi-head attention (S=2048, B=2, E=1024, H=16, D=64) on 8 Trainium2 cores.

Sharding: batch*heads head-parallel. Core c owns heads {2c, 2c+1} for both
batch elements (4 of the 32 (b,h) attention pairs). Each core:
  1. DMA-transposes x (bf16) into xT tiles [128E, S] per batch (de-interleaved).
  2. Projects q,k transposed ([col, tok]) and v natural ([tok, col]) with its
     256/130-column weight slices (q pre-scaled by D^-0.5 on host; v augmented
     with a ones column so softmax denominators fall out of the attn matmul).
  3. For each (b, h, q-chunk): scores^T tiles = k_tile^T-matmul-q (K=64),
     exp on ScalarE (PSUM->SBUF bf16), attn accumulate [128q, 65] over kpos
     (col 64 = sum of exp), then normalize with DVE reciprocal + per-partition
     scalar multiply.
Host side only slices/scales weights, casts to bf16 and concatenates outputs.
"""

import numpy as np
import ml_dtypes

S, B, E = 2048, 2, 1024
H, D = 16, 64
SCALING = D ** -0.5
NCORES = 8
SB = S * B            # 4096 tokens, row = s*B + b
HPC = H // NCORES     # 2 heads per core
KT = E // 128         # 8 contraction tiles over E
QCHUNK = 512
NQC = S // QCHUNK     # 4 q-chunks
NKT = S // 128        # 16 kpos tiles
VN = 2 * (D + 1)      # 130 v_aug cols: [v_h0(64) | 1 | v_h1(64) | 1]

_BF16 = ml_dtypes.bfloat16
_BUILT = {}


def _build_bass():
    import concourse.bacc as bacc
    import concourse.mybir as mybir
    import concourse.tile as tile
    from contextlib import ExitStack

    f32 = mybir.dt.float32
    bf = mybir.dt.bfloat16

    nc = bacc.Bacc(None, target_bir_lowering=False, debug=False)

    x_in = nc.dram_tensor("x", [SB, E], bf, kind="ExternalInput")
    wqk_in = nc.dram_tensor("wqk", [E, 256], bf, kind="ExternalInput")
    wva_in = nc.dram_tensor("wva", [E, VN], bf, kind="ExternalInput")
    bqk_in = nc.dram_tensor("bqk", [256, 1], f32, kind="ExternalInput")
    bva_in = nc.dram_tensor("bva", [1, VN], bf, kind="ExternalInput")
    out_d = nc.dram_tensor("out", [S, B, 2 * D], f32, kind="ExternalOutput")

    with tile.TileContext(nc) as tc, ExitStack() as ctx:
        const = ctx.enter_context(tc.tile_pool(name="const", bufs=1))
        res = ctx.enter_context(tc.tile_pool(name="res", bufs=1))
        expp = ctx.enter_context(tc.tile_pool(name="expp", bufs=4))
        ogp = ctx.enter_context(tc.tile_pool(name="ogp", bufs=8))
        rp = ctx.enter_context(tc.tile_pool(name="rp", bufs=8))
        ps_sc = ctx.enter_context(tc.tile_pool(name="ps_sc", bufs=2, space="PSUM"))
        ps_sm = ctx.enter_context(tc.tile_pool(name="ps_sm", bufs=4, space="PSUM"))

        # ---- constants ----
        wqk_sb = [const.tile([128, 256], bf, tag=f"wqk{k}", name=f"wqk{k}") for k in range(KT)]
        wva_sb = [const.tile([128, VN], bf, tag=f"wva{k}", name=f"wva{k}") for k in range(KT)]
        for k in range(KT):
            nc.sync.dma_start(out=wqk_sb[k][:], in_=wqk_in[k * 128:(k + 1) * 128, :])
            nc.sync.dma_start(out=wva_sb[k][:], in_=wva_in[k * 128:(k + 1) * 128, :])
        bqk_sb = const.tile([128, 2], f32, tag="bqk")
        nc.sync.dma_start(
            out=bqk_sb[:], in_=bqk_in.rearrange("(c p) o -> p (c o)", p=128)
        )
        bva_sb = const.tile([1, VN], bf, tag="bva")
        nc.sync.dma_start(out=bva_sb[:], in_=bva_in[:])
        ones_sb = const.tile([1, 128], bf, tag="ones")
        nc.vector.memset(ones_sb[:], 1.0)

        # ---- x^T via xbar DMA transpose, batch de-interleaved ----
        x3 = x_in.rearrange("(s b) e -> s b e", b=B)
        xT = [
            [res.tile([128, S], bf, tag=f"xT{b}_{k}", name=f"xT{b}_{k}") for k in range(KT)]
            for b in range(B)
        ]
        for b in range(B):
            for k in range(KT):
                nc.sync.dma_start_transpose(
                    out=xT[b][k][:], in_=x3[:, b, k * 128:(k + 1) * 128]
                )

        qT = [res.tile([128, S], bf, tag=f"qT{b}", name=f"qTt{b}") for b in range(B)]
        kT = [res.tile([128, S], bf, tag=f"kT{b}", name=f"kTt{b}") for b in range(B)]
        va = [res.tile([128, NKT, VN], bf, tag=f"va{b}", name=f"vat{b}") for b in range(B)]

        def project(b):
            # q,k transposed: out[col(2 heads x 64), tok]
            for dst, coff, boff in ((qT[b], 0, 0), (kT[b], 128, 1)):
                for t in range(NQC):
                    ps = ps_sm.tile([128, QCHUNK], f32, tag="ps1", name="projps")
                    for k in range(KT):
                        nc.tensor.matmul(
                            ps[:],
                            lhsT=wqk_sb[k][:, coff:coff + 128],
                            rhs=xT[b][k][:, t * QCHUNK:(t + 1) * QCHUNK],
                            start=(k == 0),
                            stop=(k == KT - 1),
                        )
                    nc.vector.tensor_scalar_add(
                        out=dst[:, t * QCHUNK:(t + 1) * QCHUNK],
                        in0=ps[:],
                        scalar1=bqk_sb[:, boff:boff + 1],
                    )
            # v natural with ones cols: out[tok, VN]; bias+ones via K=1 matmul
            for tt in range(NKT):
                ps = ps_sm.tile([128, VN], f32, tag="ps1", name="vps")
                nc.tensor.matmul(
                    ps[:], lhsT=ones_sb[:], rhs=bva_sb[:], start=True, stop=False
                )
                for k in range(KT):
                    nc.tensor.matmul(
                        ps[:],
                        lhsT=xT[b][k][:, tt * 128:(tt + 1) * 128],
                        rhs=wva_sb[k][:],
                        start=False,
                        stop=(k == KT - 1),
                    )
                nc.vector.tensor_copy(out=va[b][:, tt, :], in_=ps[:])

        def attend(b):
            for qc in range(NQC):
                og = [ogp.tile([128, 2 * D], f32, tag="og", name="og") for _ in range(4)]
                for h in range(HPC):
                    att = [ps_sm.tile([128, D + 1], f32, tag="ps1", name="attps") for _ in range(4)]
                    for ktp in range(NKT // 2):
                        sc = ps_sc.tile([128, 1024], f32, tag="sc", name="scps")
                        for half in range(2):
                            kt = ktp * 2 + half
                            nc.tensor.matmul(
                                sc[:, half * 512:(half + 1) * 512],
                                lhsT=kT[b][h * 64:(h + 1) * 64, kt * 128:(kt + 1) * 128],
                                rhs=qT[b][h * 64:(h + 1) * 64, qc * QCHUNK:(qc + 1) * QCHUNK],
                                start=True,
                                stop=True,
                            )
                        ex = expp.tile([128, 1024], bf, tag="ex", name="ex")
                        nc.scalar.activation(
                            out=ex[:], in_=sc[:], func=mybir.ActivationFunctionType.Exp
                        )
                        for half in range(2):
                            kt = ktp * 2 + half
                            for qs in range(4):
                                nc.tensor.matmul(
                                    att[qs][:],
                                    lhsT=ex[:, half * 512 + qs * 128:half * 512 + (qs + 1) * 128],
                                    rhs=va[b][:, kt, h * (D + 1):(h + 1) * (D + 1)],
                                    start=(kt == 0),
                                    stop=(kt == NKT - 1),
                                )
                    for qs in range(4):
                        rec = rp.tile([128, 1], f32, tag="rec", name="rec")
                        nc.vector.reciprocal(out=rec[:], in_=att[qs][:, D:D + 1])
                        nc.vector.tensor_scalar_mul(
                            out=og[qs][:, h * D:(h + 1) * D],
                            in0=att[qs][:, 0:D],
                            scalar1=rec[:],
                        )
                for qs in range(4):
                    qt = qc * 4 + qs
                    nc.sync.dma_start(
                        out=out_d[qt * 128:(qt + 1) * 128, b, :], in_=og[qs][:]
                    )

        project(0)
        attend(0)
        project(1)
        attend(1)

    nc.compile()
    return nc


def _get_nc():
    if "nc" not in _BUILT:
        _BUILT["nc"] = _build_bass()
    return _BUILT["nc"]


def _prep_core_inputs(x_bf, W, b):
    """Per-core input dicts. W/b slicing+scaling+casting is host-side weight prep."""
    in_maps = []
    for c in range(NCORES):
        q0 = 2 * c * D          # first q col of this core's head pair
        wq = W[:, q0:q0 + 128] * SCALING
        wk = W[:, E + q0:E + q0 + 128]
        wv = W[:, 2 * E + q0:2 * E + q0 + 128]
        wqk = np.concatenate([wq, wk], axis=1).astype(_BF16)
        wva = np.zeros((E, VN), np.float32)
        wva[:, 0:D] = wv[:, 0:D]
        wva[:, D + 1:2 * D + 1] = wv[:, D:2 * D]
        bqk = np.concatenate(
            [b[q0:q0 + 128] * SCALING, b[E + q0:E + q0 + 128]]
        ).astype(np.float32)[:, None]
        bva = np.zeros((1, VN), np.float32)
        bva[0, 0:D] = b[2 * E + q0:2 * E + q0 + D]
        bva[0, D] = 1.0
        bva[0, D + 1:2 * D + 1] = b[2 * E + q0 + D:2 * E + q0 + 2 * D]
        bva[0, 2 * D + 1] = 1.0
        in_maps.append(
            {
                "x": x_bf,
                "wqk": np.ascontiguousarray(wqk),
                "wva": wva.astype(_BF16),
                "bqk": np.ascontiguousarray(bqk),
                "bva": bva.astype(_BF16),
            }
        )
    return in_maps


def run(inputs, trace=False):
    """Returns (output [S,B,E] fp32, BassKernelResults)."""
    from concourse.bass_utils import run_bass_kernel_spmd

    x = np.asarray(inputs["x"], np.float32).reshape(SB, E)
    W = np.asarray(inputs["W_in"], np.float32)
    b = np.asarray(inputs["b_in"], np.float32)
    x_bf = x.astype(_BF16)

    nc = _get_nc()
    in_maps = _prep_core_inputs(x_bf, W, b)
    res = run_bass_kernel_spmd(
        nc, in_maps, core_ids=list(range(NCORES)), trace=trace
    )
    out = np.concatenate([r["out"] for r in res.results], axis=2)
    return out, res


def kernel(**inputs):
    out, _ = run(inputs, trace=False)
    return out
